# revision 8
# baseline (speedup 1.0000x reference)
"""Trainium2 Bass kernel for Ac4kAttentionOp (int8 q/k + fp8e4m3 v quantized attention).

Shapes: q,k,v [B=2, H=16, N=2048, D=64] fp32 -> out [2,16,2048,64] fp32.
Sharding: 32 (B,H) heads split 4-per-core across 8 NeuronCores; no collectives.

Math (mirrors the reference exactly up to fp32 rounding order):
  k <- k - mean_N(k)
  qq = round(q / sf_q), sf_q = max(amax_D(q)/127, eps)      (per token)
  kq = round(k / sf_k), sf_k = max(amax_D(k)/127, eps)      (per token)
  vq = fp8e4m3(v / sf_v), sf_v = max(amax_N(v)/(448/2.25), eps)  (per channel)
  s^T[m,nq] = sum_d kq[m,d] * (qq[nq,d]*sf_q[nq]*sm) ;  p^T = exp(sf_k[m] * s^T)
  outT[d,nq] = sum_m vq[m,d] * p^T[m,nq] ; denom = ones-column of vq_aug
  out[nq,d] = outT[d,nq] * sf_v[d] / denom[nq]

Performance structure:
  - ACT (exp over all N^2 scores) is the throughput floor (~133us busy/core);
    everything else is arranged to keep it fed back-to-back.
  - All main-loop matmuls (QK and PV) use 128-row fp16 stationaries: kqT/qcsT
    are zero-padded from 64 to 128 contraction rows.  Stationary row-count
    changes between consecutive matmuls serialize LDWEIGHTS (no weight
    preload) and the resulting bubbles pin the PE at its 1.2GHz mid p-state;
    uniform row counts let the PE ramp to 2.4GHz and stay there.
  - Lookahead emission: QK(mt+2)/exp(mt+2) are emitted before PV(mt) so the
    in-order PE queue always has runnable work while ACT computes exp(mt).
  - Per-head prep (quant) runs on DVE during the previous head's half-0 loop;
    prep PE transposes and the previous head's epilogue transposes are
    injected as grouped slots inside the main loops (ACT's ~2us backlog of
    queued exps covers the PE pause).
  - v-quant chain, transpose-stack copies and epilogue muls run on the
    otherwise idle Pool (GpSimd) engine; q/v loads issue from the Pool DMA
    queue so the three head loads transfer in parallel.
"""
import math
from contextlib import ExitStack

import numpy as np

import concourse.bass as bass
import concourse.tile as tile
from concourse import mybir
from concourse.masks import make_identity

B, H, N, D = 2, 16, 2048, 64
NCORES = 8
HEADS_PER_CORE = (B * H) // NCORES          # 4
SM_SCALE = 1.0 / math.sqrt(D)               # 0.125 (exact power of 2)
MAGIC = 12582912.0                          # 1.5*2^23: fp32 RNE integer round
INT8_MAX = 127.0
F8_AMAX_DIV = float(np.float32(448.0) / np.float32(2.25))  # FP8_MAX / MAX_SCALE
EPS = 1e-8

f32 = mybir.dt.float32
f16 = mybir.dt.float16
f8e4 = mybir.dt.float8e4
ALU = mybir.AluOpType
ACTF = mybir.ActivationFunctionType


def _bc(t: bass.AP, dims, off: int = 0) -> bass.AP:
    """Build a broadcast/restrided view of a tile AP (off in elements)."""
    return bass.AP(tensor=t.tensor, offset=t.offset + off, ap=dims)


def build_attention(nc: bass.Bass, heads: int = HEADS_PER_CORE, n: int = N,
                    bench_loops: int = 0):
    T = n // 128          # token tiles per head
    C = T // 2            # 128-wide transpose chunks
    NQH = n // 2          # query-half width (PSUM budget)
    q_d = nc.dram_tensor("q", [heads, n, D], f32, kind="ExternalInput").ap()
    k_d = nc.dram_tensor("k", [heads, n, D], f32, kind="ExternalInput").ap()
    v_d = nc.dram_tensor("v", [heads, n, D], f32, kind="ExternalInput").ap()
    o_d = nc.dram_tensor("out", [heads, n, D], f32, kind="ExternalOutput").ap()

    with tile.TileContext(nc) as tc, ExitStack() as ctx:
        singles = ctx.enter_context(tc.tile_pool(name="singles", bufs=1))
        loads = ctx.enter_context(tc.tile_pool(name="loads", bufs=2))
        work = ctx.enter_context(tc.tile_pool(name="work", bufs=2))
        scales = ctx.enter_context(tc.tile_pool(name="scales", bufs=2))
        small = ctx.enter_context(tc.tile_pool(name="small", bufs=4))
        opnds = ctx.enter_context(tc.tile_pool(name="opnds", bufs=2))
        pbuf = ctx.enter_context(tc.tile_pool(name="pbuf", bufs=4))
        obuf = ctx.enter_context(tc.tile_pool(name="obuf", bufs=2))
        ostore = ctx.enter_context(tc.tile_pool(name="ostore", bufs=4))
        osb = ctx.enter_context(tc.tile_pool(name="osb", bufs=2))
        ps_s = ctx.enter_context(tc.tile_pool(name="ps_s", bufs=2, space="PSUM"))
        ps_o = ctx.enter_context(tc.tile_pool(name="ps_o", bufs=1, space="PSUM"))
        ps_t = ctx.enter_context(tc.tile_pool(name="ps_t", bufs=2, space="PSUM"))

        ident_f = singles.tile([128, 128], f32)
        make_identity(nc, ident_f)
        ident_h = singles.tile([128, 128], f16)
        make_identity(nc, ident_h)
        ones_row = singles.tile([1, 128], f32)
        nc.gpsimd.memset(ones_row, 1.0)
        # constant [128,128] of 1/n in f16 (2^-11, exact): k-mean matmul weights
        invn_h = singles.tile([128, 128], f16)
        nc.gpsimd.memset(invn_h, 1.0 / n)

        if bench_loops:
            ctx.enter_context(tc.For_i(0, bench_loops, 1))

        # warm the ACT exp table before the first real exp
        warm = singles.tile([1, 1], f32)
        nc.gpsimd.memset(warm, 0.0)
        nc.scalar.activation(warm, warm, ACTF.Exp)

        def load(h):
            """k on the sync DMA queue; q and v on the gpsimd queue so the
            three transfers run in parallel.  k first: it heads the longest
            prep chain (mean -> sub -> quant -> transpose)."""
            k_sb = loads.tile([128, T, D], f32, tag="k_sb")
            nc.sync.dma_start(out=k_sb,
                              in_=k_d[h].rearrange("(t p) d -> p t d", p=128))
            q_sb = loads.tile([128, T, D], f32, tag="q_sb")
            nc.gpsimd.dma_start(out=q_sb,
                                in_=q_d[h].rearrange("(t p) d -> p t d", p=128))
            v_sb = loads.tile([128, T, D], f32, tag="v_sb")
            nc.gpsimd.dma_start(out=v_sb,
                                in_=v_d[h].rearrange("(t p) d -> p t d", p=128))
            return q_sb, k_sb, v_sb

        def prep_cast_k(bufs):
            """DVE: k -> f16 (mean-matmul moving operand)."""
            _, k_sb, _ = bufs
            k_h = work.tile([128, T, D], f16, tag="k_h")
            nc.vector.tensor_copy(k_h, k_sb)
            return k_h

        def mean_pe(k_h):
            """PE: column-mean partials via 1/n matmul (same shape family as
            the main-loop matmuls: [128,128] f16 stationary)."""
            mean_ps = ps_s.tile([128, NQH], f32, tag="pss")
            half_td = T * D // 2
            nc.tensor.matmul(mean_ps[:, 0:half_td], invn_h,
                             k_h[:, 0:T // 2, :], start=True, stop=True)
            nc.tensor.matmul(mean_ps[:, half_td:T * D], invn_h,
                             k_h[:, T // 2:T, :], start=True, stop=True)
            return mean_ps

        def quant_int8(x_sb, tagpfx):
            """per-token int8 quantize: returns (q_rounded_f32, sf [128,T])."""
            amax = scales.tile([128, T], f32, tag=tagpfx + "amax")
            nc.vector.tensor_reduce(out=amax, in_=x_sb,
                                    axis=mybir.AxisListType.X, op=ALU.max,
                                    apply_absolute_value=True)
            sf = scales.tile([128, T], f32, tag=tagpfx + "sf")
            nc.vector.tensor_scalar(out=sf, in0=amax,
                                    scalar1=1.0 / INT8_MAX, scalar2=EPS,
                                    op0=ALU.mult, op1=ALU.max)
            rsf = scales.tile([128, T], f32, tag=tagpfx + "rsf")
            nc.vector.reciprocal(rsf, sf)
            xq = work.tile([128, T, D], f32, tag=tagpfx + "xq")
            nc.vector.tensor_mul(xq, x_sb,
                                 _bc(rsf, [rsf.ap[0], [1, T], [0, D]]))
            # RNE integer round: (x + MAGIC) - MAGIC
            nc.vector.tensor_scalar(out=xq, in0=xq,
                                    scalar1=MAGIC, scalar2=MAGIC,
                                    op0=ALU.add, op1=ALU.subtract)
            return xq, sf

        def prep_k_chain(bufs, mean_ps):
            """DVE: mean reduce, mean-sub, int8 quant, f16 cast."""
            _, k_sb, _ = bufs
            meanb = small.tile([128, D], f32, tag="meanb")
            nc.vector.tensor_reduce(
                out=meanb,
                in_=_bc(mean_ps, [mean_ps.ap[0], [1, D], [D, T]]),
                axis=mybir.AxisListType.X, op=ALU.add)
            ks = work.tile([128, T, D], f32, tag="ks")
            nc.vector.tensor_sub(ks, k_sb,
                                 _bc(meanb, [meanb.ap[0], [0, T], [1, D]]))
            kq, sf_k = quant_int8(ks, "k")
            kq_h = work.tile([128, T, D], f16, tag="kq_h")
            nc.vector.tensor_copy(kq_h, kq)
            return kq_h, sf_k

        def prep_q_chain(bufs):
            """DVE: int8 quant, fold sf_q*sm into operand, f16 cast."""
            q_sb, _, _ = bufs
            qq, sf_q = quant_int8(q_sb, "q")
            csfq = scales.tile([128, T], f32, tag="csfq")
            nc.vector.tensor_scalar_mul(csfq, sf_q, SM_SCALE)
            qcs = work.tile([128, T, D], f32, tag="qcs")
            nc.vector.tensor_mul(qcs, qq,
                                 _bc(csfq, [csfq.ap[0], [1, T], [0, D]]))
            qcs_h = work.tile([128, T, D], f16, tag="qcs_h")
            nc.vector.tensor_copy(qcs_h, qcs)
            return qcs_h

        def prep_v_amax(bufs):
            """DVE: per-channel |v| max partials (channel-major view)."""
            _, _, v_sb = bufs
            amax_vp = work.tile([128, D], f32, tag="amax_vp")
            nc.vector.tensor_reduce(
                out=amax_vp,
                in_=_bc(v_sb, [v_sb.ap[0], [1, D], [D, T]]),
                axis=mybir.AxisListType.X, op=ALU.max,
                apply_absolute_value=True)
            return amax_vp

        def transpose_group(x_h, tag, queue):
            """PE chunk transposes (parity-stacked, Pool copies) then two
            strided parity-split DMAs into the top half of a zero-padded
            [128,(T,128)] operand.  queue: 'sync' or 'gpsimd' DMA queue."""
            dstT = opnds.tile([128, T, 128], f16, tag=tag)
            nc.gpsimd.memset(dstT[64:128, :, :], 0.0)
            stk = work.tile([128, C, 128], f16, tag=tag + "_st")
            for c in range(C):
                tp = ps_t.tile([128, 128], f16, tag="pst")
                nc.tensor.transpose(tp, x_h[:, 2 * c:2 * c + 2, :], ident_h)
                nc.vector.tensor_copy(stk[:, c, :], tp)
            eng = nc.sync if queue == "sync" else nc.gpsimd
            d64 = dstT[0:64]
            eng.dma_start(
                out=_bc(d64, [d64.ap[0], [2 * 128, C], [1, 128]]),
                in_=stk[0:64, :, :])
            eng.dma_start(
                out=_bc(d64, [d64.ap[0], [2 * 128, C], [1, 128]], off=128),
                in_=stk[64:128, :, :])
            return dstT

        def prep_v_scale(amax_vp, bufs):
            """PE transpose of amax partials, DVE scale math, Pool fp8 quant
            + augmented (ones-column) operand build."""
            _, _, v_sb = bufs
            vt_ps = ps_t.tile([D, 128], f32, tag="pst")
            nc.tensor.transpose(vt_ps, amax_vp, ident_f)
            amax_vT = scales.tile([D, 1], f32, tag="amax_vT")
            nc.vector.tensor_reduce(out=amax_vT, in_=vt_ps,
                                    axis=mybir.AxisListType.X, op=ALU.max)
            sf_vT = scales.tile([D, 1], f32, tag="sf_vT")
            nc.vector.tensor_scalar(out=sf_vT, in0=amax_vT,
                                    scalar1=1.0 / F8_AMAX_DIV, scalar2=EPS,
                                    op0=ALU.mult, op1=ALU.max)
            rsf_vT = scales.tile([D, 1], f32, tag="rsf_vT")
            nc.vector.reciprocal(rsf_vT, sf_vT)
            sfv65 = scales.tile([65, 1], f32, tag="sfv65")
            nc.gpsimd.memset(sfv65, 1.0)
            nc.vector.tensor_copy(sfv65[0:D, :], sf_vT)
            rsf_row = small.tile([1, D], f32, tag="rsf_row")
            nc.sync.dma_start(out=rsf_row, in_=rsf_vT)
            rsf_bps = ps_t.tile([128, D], f32, tag="pst")
            nc.tensor.matmul(rsf_bps, ones_row, rsf_row, start=True, stop=True)
            rsf_b = small.tile([128, D], f32, tag="rsf_b")
            nc.vector.tensor_copy(rsf_b, rsf_bps)
            vq_pre = work.tile([128, T, D], f32, tag="vq_pre")
            nc.gpsimd.tensor_mul(vq_pre, v_sb,
                                 _bc(rsf_b, [rsf_b.ap[0], [0, T], [1, D]]))
            vq_f8 = work.tile([128, T, D], f8e4, tag="vq_f8")
            nc.gpsimd.tensor_copy(vq_f8, vq_pre)
            vq_aug = opnds.tile([128, T, D + 1], f16, tag="vq_aug")
            nc.gpsimd.tensor_copy(vq_aug[:, :, 0:D], vq_f8)
            nc.gpsimd.memset(vq_aug[:, :, D:D + 1], 1.0)
            return vq_aug, sfv65

        def half_loop(h, st, half, slots=None):
            """Main QK->exp->PV loop for one query half (NQH queries).
            Lookahead: QK(mt+2)/exp(mt+2) emitted before PV(mt).
            slots: {mt: [closure,...]} injected after qk_exp(mt+2)."""
            slots = slots or {}
            kqT, qcsT, vq_aug = st["kqT"], st["qcsT"], st["vq_aug"]
            sf_k = st["sf_k"]
            TH = T // 2

            def qk_exp(mt):
                s_ps = ps_s.tile([128, NQH], f32, tag="pss")
                for j in range(NQH // 512):
                    rhs = qcsT[:, half * TH + 4 * j:half * TH + 4 * (j + 1), :]
                    nc.tensor.matmul(s_ps[:, j * 512:(j + 1) * 512],
                                     kqT[:, mt, :], rhs, start=True, stop=True)
                p_sb = pbuf.tile([128, NQH], f16, tag="p_sb")
                nc.scalar.activation(p_sb, s_ps, ACTF.Exp,
                                     scale=sf_k[:, mt:mt + 1])
                return p_sb

            o_ps = ps_o.tile([65, NQH], f32, tag="pso")
            ps = [qk_exp(0), qk_exp(1)]
            for mt in range(T):
                if mt + 2 < T:
                    ps.append(qk_exp(mt + 2))
                for fn in slots.get(mt, ()):
                    fn()
                p_sb = ps[mt]
                for j in range(NQH // 512):
                    nc.tensor.matmul(
                        o_ps[:, j * 512:(j + 1) * 512],
                        vq_aug[:, mt, :],
                        p_sb[:, j * 512:(j + 1) * 512],
                        start=(mt == 0), stop=(mt == T - 1))
            # scale by per-channel v scale, park in SBUF (frees the psum bank)
            outT_sb = obuf.tile([65, NQH], f32, tag="outT")
            nc.vector.tensor_scalar_mul(outT_sb, o_ps, st["sfv65"][:, 0:1])
            return outT_sb

        def epilogue(h, outTs):
            """Out-transposes + denominator divide into out_sb, then store.
            Returns a closure (PE transposes grouped) and the store closure."""
            out_sb = osb.tile([128, T, D], f32, tag="out_sb")

            def chunks():
                for half in range(2):
                    outT_sb = outTs[half]
                    for c in range(NQH // 128):
                        tp2 = ps_t.tile([128, 65], f32, tag="pst")
                        nc.tensor.transpose(
                            tp2, outT_sb[:, c * 128:(c + 1) * 128],
                            ident_f[0:65, 0:65])
                        rec = ostore.tile([128, 1], f32, tag="rec")
                        nc.vector.reciprocal(rec, tp2[:, D:D + 1])
                        nc.vector.tensor_mul(
                            out_sb[:, half * (T // 2) + c, :], tp2[:, 0:D],
                            _bc(rec, [rec.ap[0], [0, D]]))

            def store():
                nc.sync.dma_start(
                    out=o_d[h].rearrange("(t p) d -> p t d", p=128),
                    in_=out_sb)

            return chunks, store

        # ---- head pipeline ----
        # Steady-state emission for head h (engines execute in emission order,
        # per engine; data deps are tracked by Tile):
        #   half0(h):  slot2 = mean matmuls (h+1, PE);  DVE runs the full
        #              k+q quant chains of h+1 underneath.
        #   half1(h):  slot1/5 = kq/qcs transpose groups (h+1);  slot9 =
        #              v-scale (h+1);  DVE does v amax.
        #   half0(h+1): slot3 = epilogue transposes (h), then store(h).
        bufs = load(0)
        k_h = prep_cast_k(bufs)
        mean_ps = mean_pe(k_h)
        kq_h, sf_k = prep_k_chain(bufs, mean_ps)
        kqT = transpose_group(kq_h, "kqT", "sync")
        qcs_h = prep_q_chain(bufs)
        qcsT = transpose_group(qcs_h, "qcsT", "gpsimd")
        amax_vp = prep_v_amax(bufs)
        vq_aug, sfv65 = prep_v_scale(amax_vp, bufs)
        st = dict(kqT=kqT, qcsT=qcsT, vq_aug=vq_aug, sf_k=sf_k, sfv65=sfv65)

        epi_prev = None     # (chunks, store) of head h-1
        for h in range(heads):
            has_next = h + 1 < heads
            slots0, slots1 = {}, {}
            if epi_prev is not None:
                slots0[3] = [epi_prev[0], epi_prev[1]]
            st_n = {}
            if has_next:
                bufs_n = load(h + 1)
                k_hn = prep_cast_k(bufs_n)

                def s_mean(k_hn=k_hn):
                    st_n["mean_ps"] = mean_pe(k_hn)

                def s_kchain(bufs_n=bufs_n):
                    st_n["kq_h"], st_n["sf_k"] = prep_k_chain(
                        bufs_n, st_n["mean_ps"])

                def s_qchain(bufs_n=bufs_n):
                    st_n["qcs_h"] = prep_q_chain(bufs_n)

                def s_vamax(bufs_n=bufs_n):
                    st_n["amax_vp"] = prep_v_amax(bufs_n)

                slots0[2] = [s_mean, s_kchain]
                slots0[6] = [s_qchain]
                slots0[9] = [s_vamax]

                def s_kqT():
                    st_n["kqT"] = transpose_group(st_n["kq_h"], "kqT", "sync")

                def s_qcsT():
                    st_n["qcsT"] = transpose_group(st_n["qcs_h"], "qcsT",
                                                   "gpsimd")

                def s_vscale(bufs_n=bufs_n):
                    st_n["vq_aug"], st_n["sfv65"] = prep_v_scale(
                        st_n["amax_vp"], bufs_n)

                slots1[1] = [s_kqT]
                slots1[5] = [s_qcsT]
                slots1[9] = [s_vscale]
            outT0 = half_loop(h, st, 0, slots0)
            outT1 = half_loop(h, st, 1, slots1)
            epi_prev = epilogue(h, (outT0, outT1))
            if has_next:
                st = st_n
        # last head's epilogue tail
        epi_prev[0]()
        epi_prev[1]()
    return nc


_CACHED = {}


def _get_nc():
    if "nc" not in _CACHED:
        from concourse import bacc

        nc = bacc.Bacc("TRN2", target_bir_lowering=False, debug=False)
        build_attention(nc)
        nc.compile()
        _CACHED["nc"] = nc
    return _CACHED["nc"]


def kernel(q: np.ndarray, k: np.ndarray, v: np.ndarray) -> np.ndarray:
    from concourse.bass_utils import run_bass_kernel_spmd

    nc = _get_nc()
    qf = np.ascontiguousarray(np.asarray(q, dtype=np.float32).reshape(B * H, N, D))
    kf = np.ascontiguousarray(np.asarray(k, dtype=np.float32).reshape(B * H, N, D))
    vf = np.ascontiguousarray(np.asarray(v, dtype=np.float32).reshape(B * H, N, D))
    hpc = HEADS_PER_CORE
    in_maps = [
        {"q": qf[c * hpc:(c + 1) * hpc],
         "k": kf[c * hpc:(c + 1) * hpc],
         "v": vf[c * hpc:(c + 1) * hpc]}
        for c in range(NCORES)
    ]
    res = run_bass_kernel_spmd(nc, in_maps, core_ids=list(range(NCORES)))
    out = np.concatenate([np.asarray(r["out"]) for r in res.results], axis=0)
    return out.reshape(B, H, N, D).astype(np.float32)


# revision 12
# speedup vs baseline: 1.1153x; 1.1153x over previous
"""Trainium2 Bass kernel for Ac4kAttentionOp (int8 q/k + fp8e4m3 v quantized attention).

Shapes: q,k,v [B=2, H=16, N=2048, D=64] fp32 -> out [2,16,2048,64] fp32.
Sharding: 32 (B,H) heads split 4-per-core across 8 NeuronCores; no collectives.

Math (mirrors the reference exactly up to fp32 rounding order):
  k <- k - mean_N(k)
  qq = round(q / sf_q), sf_q = max(amax_D(q)/127, eps)      (per token)
  kq = round(k / sf_k), sf_k = max(amax_D(k)/127, eps)      (per token)
  vq = fp8e4m3(v / sf_v), sf_v = max(amax_N(v)/(448/2.25), eps)  (per channel)
  s^T[m,nq] = sum_d kq[m,d] * (qq[nq,d]*sf_q[nq]*sm) ;  p^T = exp(sf_k[m] * s^T)
  outT[d,nq] = sum_m vq[m,d] * p^T[m,nq] ; denom = ones-column of vq_aug
  out[nq,d] = outT[d,nq] * sf_v[d] / denom[nq]

Performance structure:
  - ACT (exp over all N^2 scores) is the throughput floor (~133us busy/core);
    everything else is arranged to keep it fed back-to-back.
  - All main-loop matmuls (QK and PV) use 128-row fp16 stationaries: kqT/qcsT
    are zero-padded from 64 to 128 contraction rows.  Stationary row-count
    changes between consecutive matmuls serialize LDWEIGHTS (no weight
    preload) and the resulting bubbles pin the PE at its 1.2GHz mid p-state;
    uniform row counts let the PE ramp to 2.4GHz and stay there.
  - Lookahead emission: QK(mt+2)/exp(mt+2) are emitted before PV(mt) so the
    in-order PE queue always has runnable work while ACT computes exp(mt).
  - Per-head prep (quant, DVE) runs during the previous head's half-0 loop;
    prep PE transposes and epilogue transposes are injected as grouped slots
    inside the main loops (ACT's ~2us backlog of queued exps covers the PE
    pause).  Head 0's k/q chains are split into token halves so the first
    QK can issue after roughly half the quant latency.
  - q/v loads and the qcsT parity-split DMAs issue from the Pool (GpSimd)
    DMA queue so transfers overlap the sync-queue ones.
"""
import math
from contextlib import ExitStack

import numpy as np

import concourse.bass as bass
import concourse.tile as tile
from concourse import mybir
from concourse.masks import make_identity

B, H, N, D = 2, 16, 2048, 64
NCORES = 8
HEADS_PER_CORE = (B * H) // NCORES          # 4
SM_SCALE = 1.0 / math.sqrt(D)               # 0.125 (exact power of 2)
MAGIC = 12582912.0                          # 1.5*2^23: fp32 RNE integer round
INT8_MAX = 127.0
F8_AMAX_DIV = float(np.float32(448.0) / np.float32(2.25))  # FP8_MAX / MAX_SCALE
EPS = 1e-8

f32 = mybir.dt.float32
f16 = mybir.dt.float16
f8e4 = mybir.dt.float8e4
ALU = mybir.AluOpType
ACTF = mybir.ActivationFunctionType


def _bc(t: bass.AP, dims, off: int = 0) -> bass.AP:
    """Build a broadcast/restrided view of a tile AP (off in elements)."""
    return bass.AP(tensor=t.tensor, offset=t.offset + off, ap=dims)


def build_attention(nc: bass.Bass, heads: int = HEADS_PER_CORE, n: int = N,
                    bench_loops: int = 0):
    T = n // 128          # token tiles per head
    C = T // 2            # 128-wide transpose chunks
    NQH = n // 2          # query-half width (PSUM budget)
    q_d = nc.dram_tensor("q", [heads, n, D], f32, kind="ExternalInput").ap()
    k_d = nc.dram_tensor("k", [heads, n, D], f32, kind="ExternalInput").ap()
    v_d = nc.dram_tensor("v", [heads, n, D], f32, kind="ExternalInput").ap()
    o_d = nc.dram_tensor("out", [heads, n, D], f32, kind="ExternalOutput").ap()

    with tile.TileContext(nc) as tc, ExitStack() as ctx:
        singles = ctx.enter_context(tc.tile_pool(name="singles", bufs=1))
        loads = ctx.enter_context(tc.tile_pool(name="loads", bufs=2))
        work = ctx.enter_context(tc.tile_pool(name="work", bufs=2))
        scales = ctx.enter_context(tc.tile_pool(name="scales", bufs=2))
        small = ctx.enter_context(tc.tile_pool(name="small", bufs=4))
        opnds = ctx.enter_context(tc.tile_pool(name="opnds", bufs=2))
        pbuf = ctx.enter_context(tc.tile_pool(name="pbuf", bufs=4))
        obuf = ctx.enter_context(tc.tile_pool(name="obuf", bufs=2))
        ostore = ctx.enter_context(tc.tile_pool(name="ostore", bufs=4))
        osb = ctx.enter_context(tc.tile_pool(name="osb", bufs=2))
        ps_s = ctx.enter_context(tc.tile_pool(name="ps_s", bufs=2, space="PSUM"))
        ps_o = ctx.enter_context(tc.tile_pool(name="ps_o", bufs=1, space="PSUM"))
        ps_t = ctx.enter_context(tc.tile_pool(name="ps_t", bufs=2, space="PSUM"))

        ident_f = singles.tile([128, 128], f32)
        make_identity(nc, ident_f)
        ident_h = singles.tile([128, 128], f16)
        make_identity(nc, ident_h)
        ones_row = singles.tile([1, 128], f32)
        nc.gpsimd.memset(ones_row, 1.0)
        # constant [128,128] of 1/n in f16 (2^-11, exact): k-mean matmul weights
        invn_h = singles.tile([128, 128], f16)
        nc.gpsimd.memset(invn_h, 1.0 / n)

        if bench_loops:
            ctx.enter_context(tc.For_i(0, bench_loops, 1))

        # warm the ACT exp table before the first real exp
        warm = singles.tile([1, 1], f32)
        nc.gpsimd.memset(warm, 0.0)
        nc.scalar.activation(warm, warm, ACTF.Exp)

        def load(h):
            """k on the sync DMA queue; q and v on the gpsimd queue so the
            three transfers run in parallel.  k first: it heads the longest
            prep chain (mean -> sub -> quant -> transpose)."""
            k_sb = loads.tile([128, T, D], f32, tag="k_sb")
            nc.sync.dma_start(out=k_sb,
                              in_=k_d[h].rearrange("(t p) d -> p t d", p=128))
            q_sb = loads.tile([128, T, D], f32, tag="q_sb")
            nc.gpsimd.dma_start(out=q_sb,
                                in_=q_d[h].rearrange("(t p) d -> p t d", p=128))
            v_sb = loads.tile([128, T, D], f32, tag="v_sb")
            nc.gpsimd.dma_start(out=v_sb,
                                in_=v_d[h].rearrange("(t p) d -> p t d", p=128))
            return q_sb, k_sb, v_sb

        def prep_cast_k(bufs, tl):
            """DVE: k -> f16 (mean-matmul moving operand)."""
            _, k_sb, _ = bufs
            tl["k_h"] = work.tile([128, T, D], f16, tag="k_h", name="k_h")
            nc.vector.tensor_copy(tl["k_h"], k_sb)

        def mean_pe(tl):
            """PE: column-mean partials via 1/n matmul (same shape family as
            the main-loop matmuls: [128,128] f16 stationary)."""
            mean_ps = ps_s.tile([128, NQH], f32, tag="pss")
            half_td = T * D // 2
            nc.tensor.matmul(mean_ps[:, 0:half_td], invn_h,
                             tl["k_h"][:, 0:T // 2, :], start=True, stop=True)
            nc.tensor.matmul(mean_ps[:, half_td:T * D], invn_h,
                             tl["k_h"][:, T // 2:T, :], start=True, stop=True)
            tl["mean_ps"] = mean_ps

        def prep_mean_red(tl):
            """DVE: reduce mean partials over token tiles."""
            meanb = small.tile([128, D], f32, tag="meanb")
            mean_ps = tl["mean_ps"]
            nc.vector.tensor_reduce(
                out=meanb,
                in_=_bc(mean_ps, [mean_ps.ap[0], [1, D], [D, T]]),
                axis=mybir.AxisListType.X, op=ALU.add)
            tl["meanb"] = meanb

        def quant_int8(x_sb, tagpfx, tl, t0, t1):
            """per-token int8 quantize of tiles [t0,t1); scale/stage tiles in
            tl are allocated on the first part, sub-written on later parts."""
            nt = t1 - t0
            key = tagpfx + "amax"
            if key not in tl:
                tl[key] = scales.tile([128, T], f32, tag=key, name=key)
                tl[tagpfx + "sf"] = scales.tile([128, T], f32,
                                                tag=tagpfx + "sf",
                                                name=tagpfx + "sf")
                tl[tagpfx + "rsf"] = scales.tile([128, T], f32,
                                                 tag=tagpfx + "rsf",
                                                 name=tagpfx + "rsf")
                tl[tagpfx + "xq"] = work.tile([128, T, D], f32,
                                              tag=tagpfx + "xq",
                                              name=tagpfx + "xq")
            amax, sf = tl[key], tl[tagpfx + "sf"]
            rsf, xq = tl[tagpfx + "rsf"], tl[tagpfx + "xq"]
            nc.vector.tensor_reduce(out=amax[:, t0:t1], in_=x_sb[:, t0:t1, :],
                                    axis=mybir.AxisListType.X, op=ALU.max,
                                    apply_absolute_value=True)
            nc.vector.tensor_scalar(out=sf[:, t0:t1], in0=amax[:, t0:t1],
                                    scalar1=1.0 / INT8_MAX, scalar2=EPS,
                                    op0=ALU.mult, op1=ALU.max)
            nc.vector.reciprocal(rsf[:, t0:t1], sf[:, t0:t1])
            nc.vector.tensor_mul(
                xq[:, t0:t1, :], x_sb[:, t0:t1, :],
                _bc(rsf, [rsf.ap[0], [1, nt], [0, D]], off=t0))
            # RNE integer round: (x + MAGIC) - MAGIC
            nc.vector.tensor_scalar(out=xq[:, t0:t1, :], in0=xq[:, t0:t1, :],
                                    scalar1=MAGIC, scalar2=MAGIC,
                                    op0=ALU.add, op1=ALU.subtract)

        def prep_k_chain(bufs, tl, t0=0, t1=None):
            """DVE: mean-sub + int8 quant + f16 cast for k tiles [t0,t1)."""
            _, k_sb, _ = bufs
            t1 = T if t1 is None else t1
            nt = t1 - t0
            if "ks" not in tl:
                tl["ks"] = work.tile([128, T, D], f32, tag="ks", name="ks")
                tl["kq_h"] = work.tile([128, T, D], f16, tag="kq_h", name="kq_h")
            meanb = tl["meanb"]
            nc.vector.tensor_sub(tl["ks"][:, t0:t1, :], k_sb[:, t0:t1, :],
                                 _bc(meanb, [meanb.ap[0], [0, nt], [1, D]]))
            quant_int8(tl["ks"], "k", tl, t0, t1)
            nc.vector.tensor_copy(tl["kq_h"][:, t0:t1, :],
                                  tl["kxq"][:, t0:t1, :])

        def prep_q_chain(bufs, tl, t0=0, t1=None):
            """DVE: int8 quant + fold sf_q*sm + f16 cast for q tiles."""
            q_sb, _, _ = bufs
            t1 = T if t1 is None else t1
            nt = t1 - t0
            if "qcs_h" not in tl:
                tl["csfq"] = scales.tile([128, T], f32, tag="csfq", name="csfq")
                tl["qcs"] = work.tile([128, T, D], f32, tag="qcs", name="qcs")
                tl["qcs_h"] = work.tile([128, T, D], f16, tag="qcs_h", name="qcs_h")
            quant_int8(q_sb, "q", tl, t0, t1)
            csfq = tl["csfq"]
            nc.vector.tensor_scalar_mul(csfq[:, t0:t1], tl["qsf"][:, t0:t1],
                                        SM_SCALE)
            nc.vector.tensor_mul(
                tl["qcs"][:, t0:t1, :], tl["qxq"][:, t0:t1, :],
                _bc(csfq, [csfq.ap[0], [1, nt], [0, D]], off=t0))
            nc.vector.tensor_copy(tl["qcs_h"][:, t0:t1, :],
                                  tl["qcs"][:, t0:t1, :])

        def prep_v_amax(bufs, tl):
            """DVE: per-channel |v| max partials (channel-major view)."""
            _, _, v_sb = bufs
            amax_vp = work.tile([128, D], f32, tag="amax_vp")
            nc.vector.tensor_reduce(
                out=amax_vp,
                in_=_bc(v_sb, [v_sb.ap[0], [1, D], [D, T]]),
                axis=mybir.AxisListType.X, op=ALU.max,
                apply_absolute_value=True)
            tl["amax_vp"] = amax_vp

        def transpose_group(src_key, dst_key, tag, queue, tl, c0=0, c1=None):
            """PE chunk transposes (parity-stacked via DVE) of chunks [c0,c1)
            then two strided parity-split DMAs into the top half of the
            zero-padded [128,(T,128)] operand."""
            c1 = C if c1 is None else c1
            if dst_key not in tl:
                tl[dst_key] = opnds.tile([128, T, 128], f16, tag=tag, name=tag)
                nc.gpsimd.memset(tl[dst_key][64:128, :, :], 0.0)
                tl[dst_key + "_st"] = work.tile([128, C, 128], f16,
                                                tag=tag + "_st",
                                                name=tag + "_st")
            dstT, stk = tl[dst_key], tl[dst_key + "_st"]
            x_h = tl[src_key]
            for c in range(c0, c1):
                tp = ps_t.tile([128, 128], f16, tag="pst")
                nc.tensor.transpose(tp, x_h[:, 2 * c:2 * c + 2, :], ident_h)
                nc.vector.tensor_copy(stk[:, c, :], tp)
            eng = nc.sync if queue == "sync" else nc.gpsimd
            d64 = dstT[0:64]
            nci = c1 - c0
            eng.dma_start(
                out=_bc(d64, [d64.ap[0], [2 * 128, nci], [1, 128]],
                        off=c0 * 256),
                in_=stk[0:64, c0:c1, :])
            eng.dma_start(
                out=_bc(d64, [d64.ap[0], [2 * 128, nci], [1, 128]],
                        off=c0 * 256 + 128),
                in_=stk[64:128, c0:c1, :])

        def prep_v_scale(bufs, tl):
            """PE transpose of amax partials, DVE scale math + fp8 quant +
            augmented (ones-column) operand build."""
            _, _, v_sb = bufs
            vt_ps = ps_t.tile([D, 128], f32, tag="pst")
            nc.tensor.transpose(vt_ps, tl["amax_vp"], ident_f)
            amax_vT = scales.tile([D, 1], f32, tag="amax_vT")
            nc.vector.tensor_reduce(out=amax_vT, in_=vt_ps,
                                    axis=mybir.AxisListType.X, op=ALU.max)
            sf_vT = scales.tile([D, 1], f32, tag="sf_vT")
            nc.vector.tensor_scalar(out=sf_vT, in0=amax_vT,
                                    scalar1=1.0 / F8_AMAX_DIV, scalar2=EPS,
                                    op0=ALU.mult, op1=ALU.max)
            rsf_vT = scales.tile([D, 1], f32, tag="rsf_vT")
            nc.vector.reciprocal(rsf_vT, sf_vT)
            sfv65 = scales.tile([65, 1], f32, tag="sfv65")
            nc.gpsimd.memset(sfv65, 1.0)
            nc.vector.tensor_copy(sfv65[0:D, :], sf_vT)
            rsf_row = small.tile([1, D], f32, tag="rsf_row")
            nc.sync.dma_start(out=rsf_row, in_=rsf_vT)
            rsf_bps = ps_t.tile([128, D], f32, tag="pst")
            nc.tensor.matmul(rsf_bps, ones_row, rsf_row, start=True, stop=True)
            rsf_b = small.tile([128, D], f32, tag="rsf_b")
            nc.vector.tensor_copy(rsf_b, rsf_bps)
            vq_pre = work.tile([128, T, D], f32, tag="vq_pre")
            nc.vector.tensor_mul(vq_pre, v_sb,
                                 _bc(rsf_b, [rsf_b.ap[0], [0, T], [1, D]]))
            vq_f8 = work.tile([128, T, D], f8e4, tag="vq_f8")
            nc.vector.tensor_copy(vq_f8, vq_pre)
            vq_aug = opnds.tile([128, T, D + 1], f16, tag="vq_aug")
            nc.vector.tensor_copy(vq_aug[:, :, 0:D], vq_f8)
            nc.gpsimd.memset(vq_aug[:, :, D:D + 1], 1.0)
            tl["vq_aug"] = vq_aug
            tl["sfv65"] = sfv65

        def half_loop(h, tl, half, slots=None):
            """Main QK->exp->PV loop for one query half (NQH queries).
            Lookahead: QK(mt+2)/exp(mt+2) emitted before PV(mt).
            slots: {mt: [closure,...]} run after qk_exp(mt+2) is emitted."""
            slots = slots or {}
            kqT, qcsT, vq_aug = tl["kqT"], tl["qcsT"], tl["vq_aug"]
            sf_k = tl["ksf"]
            TH = T // 2

            def qk_exp(mt):
                s_ps = ps_s.tile([128, NQH], f32, tag="pss")
                for j in range(NQH // 512):
                    rhs = qcsT[:, half * TH + 4 * j:half * TH + 4 * (j + 1), :]
                    nc.tensor.matmul(s_ps[:, j * 512:(j + 1) * 512],
                                     kqT[:, mt, :], rhs, start=True, stop=True)
                p_sb = pbuf.tile([128, NQH], f16, tag="p_sb")
                nc.scalar.activation(p_sb, s_ps, ACTF.Exp,
                                     scale=sf_k[:, mt:mt + 1])
                return p_sb

            o_ps = ps_o.tile([65, NQH], f32, tag="pso")
            ps = [qk_exp(0), qk_exp(1)]
            for mt in range(T):
                if mt + 2 < T:
                    ps.append(qk_exp(mt + 2))
                for fn in slots.get(mt, ()):
                    fn()
                p_sb = ps[mt]
                for j in range(NQH // 512):
                    nc.tensor.matmul(
                        o_ps[:, j * 512:(j + 1) * 512],
                        vq_aug[:, mt, :],
                        p_sb[:, j * 512:(j + 1) * 512],
                        start=(mt == 0), stop=(mt == T - 1))
            # scale by per-channel v scale, park in SBUF (frees the psum bank)
            outT_sb = obuf.tile([65, NQH], f32, tag="outT")
            nc.vector.tensor_scalar_mul(outT_sb, o_ps, tl["sfv65"][:, 0:1])
            return outT_sb

        def epilogue_half(outT_sb, out_sb, half):
            """Out-transposes + denominator divide for one query half."""
            def chunks():
                for c in range(NQH // 128):
                    tp2 = ps_t.tile([128, 65], f32, tag="pst")
                    nc.tensor.transpose(tp2, outT_sb[:, c * 128:(c + 1) * 128],
                                        ident_f[0:65, 0:65])
                    rec = ostore.tile([128, 1], f32, tag="rec")
                    nc.vector.reciprocal(rec, tp2[:, D:D + 1])
                    nc.vector.tensor_mul(
                        out_sb[:, half * (T // 2) + c, :], tp2[:, 0:D],
                        _bc(rec, [rec.ap[0], [0, D]]))
            return chunks

        # ---- head pipeline ----
        # Steady-state emission for head h:
        #   half0(h): slot2 = mean matmuls (h+1, PE) + epilogue chunks of
        #             h-1 half1 + store(h-1); DVE runs the k+q+v quant
        #             chains of h+1 underneath (emitted in slots).
        #   half1(h): slot1/5 = kq/qcs transpose groups (h+1);
        #             slot3 = epilogue chunks of h half0; slot9 = v-scale.
        # Head 0: k/q chains split into token halves so QK(0) issues early.
        tl = {}
        bufs = load(0)
        prep_cast_k(bufs, tl)
        mean_pe(tl)
        prep_mean_red(tl)
        TH2 = T // 2
        prep_k_chain(bufs, tl, 0, TH2)
        transpose_group("kq_h", "kqT", "kqT", "sync", tl, 0, C // 2)
        prep_q_chain(bufs, tl, 0, TH2)
        transpose_group("qcs_h", "qcsT", "qcsT", "gpsimd", tl, 0, C // 2)
        prep_v_amax(bufs, tl)
        prep_v_scale(bufs, tl)

        def k_part2(bufs=bufs, tl=tl):
            prep_k_chain(bufs, tl, TH2, T)

        def kT_part2(tl=tl):
            transpose_group("kq_h", "kqT", "kqT", "sync", tl, C // 2, C)

        def q_part2(bufs=bufs, tl=tl):
            prep_q_chain(bufs, tl, TH2, T)

        def qT_part2(tl=tl):
            transpose_group("qcs_h", "qcsT", "qcsT", "gpsimd", tl, C // 2, C)

        h0_slots0 = {0: [k_part2], 2: [kT_part2], 4: [q_part2],
                     6: [qT_part2]}

        prev_chunks1 = None    # epilogue closure: half1 of previous head
        prev_store = None
        for h in range(heads):
            has_next = h + 1 < heads
            out_sb = osb.tile([128, T, D], f32, tag="out_sb")
            slots0 = dict(h0_slots0) if h == 0 else {}
            h0_slots0 = {}
            if prev_chunks1 is not None:
                slots0.setdefault(3, []).append(prev_chunks1)
                slots0.setdefault(3, []).append(prev_store)
            tl_n = {}
            if has_next:
                bufs_n = load(h + 1)

                def s_cast(bufs_n=bufs_n, tl_n=tl_n):
                    prep_cast_k(bufs_n, tl_n)

                def s_mean(tl_n=tl_n):
                    mean_pe(tl_n)
                    prep_mean_red(tl_n)

                def s_kchain(bufs_n=bufs_n, tl_n=tl_n):
                    prep_k_chain(bufs_n, tl_n)

                def s_qchain(bufs_n=bufs_n, tl_n=tl_n):
                    prep_q_chain(bufs_n, tl_n)

                def s_vamax(bufs_n=bufs_n, tl_n=tl_n):
                    prep_v_amax(bufs_n, tl_n)

                slots0.setdefault(1, []).append(s_cast)
                slots0.setdefault(2, []).append(s_mean)
                slots0.setdefault(4, []).append(s_kchain)
                slots0.setdefault(7, []).append(s_qchain)
                slots0.setdefault(10, []).append(s_vamax)
            outT0 = half_loop(h, tl, 0, slots0)
            slots1 = {}
            slots1[3] = [epilogue_half(outT0, out_sb, 0)]
            if has_next:
                def s_kqT(tl_n=tl_n):
                    transpose_group("kq_h", "kqT", "kqT", "sync", tl_n)

                def s_qcsT(tl_n=tl_n):
                    transpose_group("qcs_h", "qcsT", "qcsT", "gpsimd", tl_n)

                def s_vscale(bufs_n=bufs_n, tl_n=tl_n):
                    prep_v_scale(bufs_n, tl_n)

                slots1[1] = [s_kqT]
                slots1[6] = [s_qcsT]
                slots1[9] = [s_vscale]
            outT1 = half_loop(h, tl, 1, slots1)
            prev_chunks1 = epilogue_half(outT1, out_sb, 1)

            def prev_store(h=h, out_sb=out_sb):
                nc.sync.dma_start(
                    out=o_d[h].rearrange("(t p) d -> p t d", p=128),
                    in_=out_sb)
            if has_next:
                tl = tl_n
        # last head's half-1 epilogue tail
        prev_chunks1()
        prev_store()
    return nc


_CACHED = {}


def _get_nc():
    if "nc" not in _CACHED:
        from concourse import bacc

        nc = bacc.Bacc("TRN2", target_bir_lowering=False, debug=False)
        build_attention(nc)
        nc.compile()
        _CACHED["nc"] = nc
    return _CACHED["nc"]


def kernel(q: np.ndarray, k: np.ndarray, v: np.ndarray) -> np.ndarray:
    from concourse.bass_utils import run_bass_kernel_spmd

    nc = _get_nc()
    qf = np.ascontiguousarray(np.asarray(q, dtype=np.float32).reshape(B * H, N, D))
    kf = np.ascontiguousarray(np.asarray(k, dtype=np.float32).reshape(B * H, N, D))
    vf = np.ascontiguousarray(np.asarray(v, dtype=np.float32).reshape(B * H, N, D))
    hpc = HEADS_PER_CORE
    in_maps = [
        {"q": qf[c * hpc:(c + 1) * hpc],
         "k": kf[c * hpc:(c + 1) * hpc],
         "v": vf[c * hpc:(c + 1) * hpc]}
        for c in range(NCORES)
    ]
    res = run_bass_kernel_spmd(nc, in_maps, core_ids=list(range(NCORES)))
    out = np.concatenate([np.asarray(r["out"]) for r in res.results], axis=0)
    return out.reshape(B, H, N, D).astype(np.float32)
